# revision 11
# baseline (speedup 1.0000x reference)
"""MultiHeadAttention (B=2, T=2048, D=512, H=8, causal) on 8 trn2 NeuronCores.

Sharding: batch*heads across cores. Core c handles batch c//4 and heads
{2*(c%4), 2*(c%4)+1}. Each core projects Q/K/V for its two heads (weight
slices replicated), runs softmax attention with scores materialized
transposed ([keys, queries] so the softmax reduction lands on the PSUM/matmul
path instead of cross-partition ops), applies its slice of the output
projection, and writes a [T, D] partial. Host sums the 8 partials (+ output
bias) into the full [B, T, D] result.

Numerics: all large matmuls run in float32r (TF32-like, ~1.2e-4 relative),
exact-fp32 PE transposes for V, softmax without max-subtraction (scores are
O(1) by construction: Q/K projections of unit-variance data through
U(-1/sqrt(D)) weights; exp stays far from fp32 range).
"""

import numpy as np

import concourse.bass as bass
import concourse.mybir as mybir
import concourse.tile as tile
from concourse.bass_utils import run_bass_kernel_spmd

D_MODEL = 512
N_HEADS = 8
D_K = 64
B = 2
T = 2048
N_CORES = 8
P = 128
QC = 4           # query chunks of 512
QW = T // QC     # 512 queries per chunk
KT = T // P      # 16 key tiles of 128
F32 = mybir.dt.float32
F32R = mybir.dt.float32r
BF16 = mybir.dt.bfloat16

# dtype configuration (module-level; set before first kernel() call)
ATTN_DT = "f32r"   # "f32r" | "bf16"  -- scores/AV operand dtype
IN_DT = "f32"      # "f32" | "bf16"   -- host->device x + projection dtype

_BUILD_CACHE = {}


def _hoist_excess_waits(nc, max_waits=1):
    """walrus codegen supports at most one sync-wait slot per hardware
    instruction, but Tile's sem-assignment can attach several (e.g. inputs
    arriving via two HW-DGE queues). Move the excess onto same-engine no-ops
    placed just before the instruction."""
    n_fixed = 0
    for fn in nc.m.functions:
        for bb in fn.blocks:
            insts = bb.instructions
            new_list = []
            for ins in insts:
                si = ins.sync_info
                ow = list(si.on_wait or []) if si else []
                if len(ow) > max_waits and ins.is_executable():
                    for j, w in enumerate(ow[max_waits:]):
                        nop = mybir.InstNoOp(
                            name=f"waitnop{j}_{ins.name}", ins=[], outs=[]
                        )
                        nop.engine = ins.engine
                        nop.sync_info = mybir.SyncInfo(on_wait=[w], on_update=[])
                        new_list.append(nop)
                    si.on_wait = ow[:max_waits]
                    ins.sync_info = si
                    n_fixed += 1
                new_list.append(ins)
            insts[:] = new_list
    return n_fixed


def _build_nc(plan, attn_dt, in_dt):
    """plan: 'causal' or 'full' -> one SPMD program for all 8 cores."""
    nc = bass.Bass(target_bir_lowering=False)
    ADT = BF16 if attn_dt == "bf16" else F32R
    XDT = BF16 if in_dt == "bf16" else F32
    XSB = BF16 if in_dt == "bf16" else F32R

    qT = nc.dram_tensor("qT", [P, 4, T], XDT, kind="ExternalInput")
    kT = nc.dram_tensor("kT", [P, 4, T], XDT, kind="ExternalInput")
    vT = nc.dram_tensor("vT", [P, 4, T], XDT, kind="ExternalInput")
    # wqkv: [128, 4(kc), 3(q/k/v), 128] interleaved on host
    wqkv = nc.dram_tensor("wqkv", [P, 4, 3, P], XDT, kind="ExternalInput")
    # misc: [128, 3+64+64] = biases (q,k,v) | idstack | ones
    misc = nc.dram_tensor("misc", [P, 3 + 2 * D_K], F32, kind="ExternalInput")
    # wo2: [128, 512] -- both heads' wo columns stacked on partitions
    wo2 = nc.dram_tensor("wo2", [P, D_MODEL], F32, kind="ExternalInput")
    outp = nc.dram_tensor("outp", [T, D_MODEL], F32, kind="ExternalOutput")

    def kts_of(qc):
        return list(range(KT if plan == "full" else 4 * (qc + 1)))

    def is_partial(qc, kt):
        return plan == "causal" and 4 * qc <= kt <= 4 * qc + 3

    with tile.TileContext(nc) as tc:
        with (
            tc.tile_pool(name="consts", bufs=1) as consts,
            tc.tile_pool(name="xin", bufs=1) as xin,
            tc.tile_pool(name="projT", bufs=1) as projT,
            tc.tile_pool(name="epool", bufs=6) as epool,
            tc.tile_pool(name="rpool", bufs=2) as rpool,
            tc.tile_pool(name="opool", bufs=3) as opool,
            tc.tile_pool(name="mm_ps", bufs=1, space="PSUM") as mm_ps,
            tc.tile_pool(name="s2_ps", bufs=2, space="PSUM") as s2_ps,
            tc.tile_pool(name="ctx_ps", bufs=3, space="PSUM") as ctx_ps_pool,
        ):
            # ---- constants (3 consolidated DMAs) ----
            wqkv_sb = consts.tile([P, 4, 3, P], XSB, tag="wqkv")
            nc.scalar.dma_start(wqkv_sb, wqkv.ap().bitcast(XSB))
            wq_sb = wqkv_sb[:, :, 0, :]
            wk_sb = wqkv_sb[:, :, 1, :]
            wv_sb = wqkv_sb[:, :, 2, :]
            misc_sb = consts.tile([P, 3 + 2 * D_K], F32, tag="misc")
            nc.sync.dma_start(misc_sb, misc[:, :])
            bq_sb = misc_sb[:, 0:1]
            bk_sb = misc_sb[:, 1:2]
            bv_sb = misc_sb[:, 2:3]
            ident = misc_sb[:, 3:3 + D_K]
            ones_r = consts.tile([P, D_K], F32R, tag="ones_r")
            nc.scalar.dma_start(ones_r, misc[:, 3 + D_K:3 + 2 * D_K].bitcast(F32R))
            wo_all = consts.tile([P, D_MODEL], F32R, tag="wo")
            nc.scalar.dma_start(wo_all, wo2.ap().bitcast(F32R))

            # ---- projections ----
            qhT = projT.tile([P, T], ADT, tag="qhT")   # [2*dk, T]
            khT = projT.tile([P, T], ADT, tag="khT")
            vhT = projT.tile([P, T], F32, tag="vhT")    # fp32: transposed exactly

            vaug = projT.tile([P, KT, 2, D_K + 1], ADT, tag="vaug")

            for h in range(2):
                nc.vector.tensor_copy(
                    vaug[:, :, h, D_K:D_K + 1],
                    ones_r[:, 0:1, None].to_broadcast((P, KT, 1)),
                )

            x_tiles = {}

            def load_x(xT_dram, xname):
                x_sb = xin.tile([P, 4, T], XSB, tag=f"x_{xname}", name=f"x_{xname}")
                for kc in range(4):
                    eng = nc.sync if kc % 2 == 0 else nc.scalar
                    eng.dma_start(
                        x_sb[:, kc, :], xT_dram.ap().bitcast(XSB)[:, kc, :]
                    )
                x_tiles[xname] = x_sb

            def project_chunk(xname, w_sb, b_sb, dst, qc):
                x_sb = x_tiles[xname]
                ps = mm_ps.tile([P, QW], F32, tag="mm", name=f"ps_{xname}_{qc}")
                for kc in range(4):
                    nc.tensor.matmul(
                        ps, w_sb[:, kc, :], x_sb[:, kc, qc * QW:(qc + 1) * QW],
                        start=(kc == 0), stop=(kc == 3),
                    )
                nc.vector.tensor_scalar(
                    out=dst[:, qc * QW:(qc + 1) * QW],
                    in0=ps, scalar1=b_sb, scalar2=None,
                    op0=mybir.AluOpType.add,
                )

            def vaug_chunk(qc):
                # transpose v-chunk qc's 4 key tiles into vaug (exact fp32)
                for kt in range(4 * qc, 4 * qc + 4):
                    for h in range(2):
                        tr = mm_ps.tile([P, D_K], F32, tag="mm", name=f"tr_{kt}_{h}")
                        nc.tensor.transpose(
                            tr,
                            vhT[h * D_K:(h + 1) * D_K, kt * P:(kt + 1) * P],
                            ident[h * D_K:(h + 1) * D_K, :],
                        )
                        nc.vector.tensor_copy(vaug[:, kt, h, 0:D_K], tr)

            # load whole tensors (128 x 8KB descriptors per DMA), then project
            load_x(kT, "k")
            load_x(qT, "q")
            load_x(vT, "v")
            for qc in range(QC):
                project_chunk("k", wk_sb, bk_sb, khT, qc)
            project_chunk("q", wq_sb, bq_sb, qhT, 3)
            for qc in range(QC):
                project_chunk("v", wv_sb, bv_sb, vhT, qc)
                vaug_chunk(qc)
            for qc in (2, 1, 0):
                project_chunk("q", wq_sb, bq_sb, qhT, qc)

            # ---- attention + output projection ----
            ctxT = projT.tile([P, T], F32R, tag="ctxT")

            for qc in (3, 2, 1, 0):
                kts = kts_of(qc)
                ctx_ps = [
                    ctx_ps_pool.tile([D_K + 1, QW], F32, tag="ctx", name=f"ctx_{qc}_{h}")
                    for h in range(2)
                ]
                def q_lo(kt):
                    # first valid query column within this chunk (diag trimming)
                    if plan != "causal":
                        return 0
                    return max(0, kt * P - qc * QW)

                for kp in range(0, len(kts), 2):
                    pair = kts[kp:kp + 2]
                    for h in range(2):
                        hs = slice(h * D_K, (h + 1) * D_K)
                        s_ps = s2_ps.tile([P, 2, QW], F32, tag="s2")
                        for j, kt in enumerate(pair):
                            lo = q_lo(kt)
                            nc.tensor.matmul(
                                s_ps[:, j, lo:],
                                khT[hs, kt * P:(kt + 1) * P],
                                qhT[hs, qc * QW + lo:(qc + 1) * QW],
                                start=True, stop=True,
                            )
                        e_sb = epool.tile([P, 2, QW], ADT, tag="e")
                        lo0 = q_lo(pair[0])
                        if len(pair) == 2 and q_lo(pair[1]) == lo0:
                            nc.scalar.activation(
                                e_sb[:, :, lo0:], s_ps[:, :, lo0:],
                                mybir.ActivationFunctionType.Exp,
                            )
                        else:
                            for j, kt in enumerate(pair):
                                lo = q_lo(kt)
                                nc.scalar.activation(
                                    e_sb[:, j, lo:], s_ps[:, j, lo:],
                                    mybir.ActivationFunctionType.Exp,
                                )
                        for j, kt in enumerate(pair):
                            lo = q_lo(kt)
                            if is_partial(qc, kt):
                                nc.gpsimd.affine_select(
                                    out=e_sb[:, j, lo:], in_=e_sb[:, j, lo:],
                                    compare_op=mybir.AluOpType.is_ge,
                                    fill=0.0,
                                    base=qc * QW + lo - kt * P,
                                    pattern=[[1, QW - lo]],
                                    channel_multiplier=-1,
                                )
                            nc.tensor.matmul(
                                ctx_ps[h][:, lo:],
                                vaug[:, kt, h, :],
                                e_sb[:, j, lo:],
                                start=(kt == kts[0]), stop=(kt == kts[-1]),
                            )
                for h in range(2):
                    recip = rpool.tile([D_K + 1, QW], F32R, tag="recip")
                    lnd = rpool.tile([D_K + 1, QW], F32, tag="lnd")
                    nc.scalar.activation(
                        lnd[D_K:D_K + 1, :], ctx_ps[h][D_K:D_K + 1, :],
                        mybir.ActivationFunctionType.Ln,
                    )
                    nc.scalar.activation(
                        recip[D_K:D_K + 1, :], lnd[D_K:D_K + 1, :],
                        mybir.ActivationFunctionType.Exp, scale=-1.0,
                    )
                    bc_ps = mm_ps.tile([D_K, QW], F32, tag="mm")
                    nc.tensor.matmul(
                        bc_ps,
                        ones_r[D_K:D_K + 1, :],
                        recip[D_K:D_K + 1, :],
                        start=True, stop=True,
                    )
                    bcast = rpool.tile([D_K, QW], F32R, tag="bcast")
                    nc.vector.tensor_copy(bcast, bc_ps)
                    nc.vector.tensor_tensor(
                        ctxT[h * D_K:(h + 1) * D_K, qc * QW:(qc + 1) * QW],
                        ctx_ps[h][0:D_K, :],
                        bcast,
                        mybir.AluOpType.mult,
                    )

                # O-projection for this query chunk (4 token tiles of 128)
                for qt in range(qc * 4, (qc + 1) * 4):
                    o_ps = mm_ps.tile([P, D_MODEL], F32, tag="mm")
                    nc.tensor.matmul(
                        o_ps, ctxT[:, qt * P:(qt + 1) * P], wo_all,
                        start=True, stop=True,
                    )
                    o_sb = opool.tile([P, D_MODEL], F32, tag="o")
                    nc.vector.tensor_copy(o_sb, o_ps)
                    nc.sync.dma_start(outp[qt * P:(qt + 1) * P, :], o_sb)

    _hoist_excess_waits(nc)
    return nc


def get_nc(plan):
    key = (plan, ATTN_DT, IN_DT)
    if key not in _BUILD_CACHE:
        _BUILD_CACHE[key] = _build_nc(plan, ATTN_DT, IN_DT)
    return _BUILD_CACHE[key]


def make_in_maps(q, k, v, wq, bq, wk, bk, wv, bv, wo):
    scale = 1.0 / np.sqrt(D_K)
    if IN_DT == "bf16":
        import ml_dtypes
        xdt = ml_dtypes.bfloat16
    else:
        xdt = np.float32
    idstack = np.concatenate([np.eye(D_K, dtype=np.float32)] * 2, axis=0)
    ones_in = np.ones((P, D_K), dtype=np.float32)
    def interleave(x):
        # [T, D] -> x.T grouped as [128, 4, T]: row kc*128+p of x.T at [p, kc]
        return np.ascontiguousarray(
            x.T.reshape(4, P, T).transpose(1, 0, 2)
        ).astype(xdt)

    def w_interleave(w):
        # [128 out, 512 in] -> lhsT chunks [128 p, 4 kc, 128 out]
        return w.T.reshape(4, P, P).transpose(1, 0, 2)

    in_maps = []
    for c in range(N_CORES):
        b = c // (N_CORES // B)
        h0 = 2 * (c % (N_CORES // B))
        ds = slice(h0 * D_K, (h0 + 2) * D_K)
        wqkv_arr = np.ascontiguousarray(np.stack([
            w_interleave(wq[ds] * scale),
            w_interleave(wk[ds]),
            w_interleave(wv[ds]),
        ], axis=2)).astype(xdt)
        misc_arr = np.ascontiguousarray(np.concatenate([
            (bq[ds] * scale).reshape(P, 1),
            bk[ds].reshape(P, 1),
            bv[ds].reshape(P, 1),
            idstack,
            ones_in,
        ], axis=1)).astype(np.float32)
        in_maps.append({
            "qT": interleave(q[b]),
            "kT": interleave(k[b]),
            "vT": interleave(v[b]),
            "wqkv": wqkv_arr,
            "misc": misc_arr,
            "wo2": np.ascontiguousarray(wo[:, ds].T),
        })
    return in_maps


def classify_mask(mask):
    m = np.asarray(mask)
    if m.all():
        return "full"
    tril = np.tril(np.ones((T, T), dtype=bool))
    if all(np.array_equal(m[b, 0], tril) for b in range(m.shape[0])):
        return "causal"
    return "general"


def _numpy_reference(q, k, v, mask, wq, bq, wk, bk, wv, bv, wo, bo):
    """Fallback for mask patterns the device program doesn't cover."""
    qh = (q @ wq.T + bq).reshape(B, T, N_HEADS, D_K).transpose(0, 2, 1, 3)
    kh = (k @ wk.T + bk).reshape(B, T, N_HEADS, D_K).transpose(0, 2, 1, 3)
    vh = (v @ wv.T + bv).reshape(B, T, N_HEADS, D_K).transpose(0, 2, 1, 3)
    s = np.einsum("bhqd,bhkd->bhqk", qh, kh) / np.sqrt(D_K).astype(np.float32)
    s = np.where(mask, s, -np.inf)
    all_masked = ~mask.any(axis=-1, keepdims=True)
    s = np.where(all_masked, 0.0, s)
    s = s - s.max(axis=-1, keepdims=True)
    e = np.exp(s)
    p = e / e.sum(axis=-1, keepdims=True)
    ctx = np.einsum("bhqk,bhkd->bhqd", p, vh)
    ctx = ctx.transpose(0, 2, 1, 3).reshape(B, T, D_MODEL)
    return (ctx @ wo.T + bo).astype(np.float32)


def kernel(q, k, v, mask, wq, bq, wk, bk, wv, bv, wo, bo, _trace=False):
    q, k, v = (np.asarray(x, dtype=np.float32) for x in (q, k, v))
    mask = np.asarray(mask, dtype=bool)
    wq, bq, wk, bk, wv, bv, wo, bo = (
        np.asarray(x, dtype=np.float32) for x in (wq, bq, wk, bk, wv, bv, wo, bo)
    )

    plan = classify_mask(mask)
    if plan == "general":
        return _numpy_reference(q, k, v, mask, wq, bq, wk, bk, wv, bv, wo, bo)

    nc = get_nc(plan)
    in_maps = make_in_maps(q, k, v, wq, bq, wk, bk, wv, bv, wo)
    res = run_bass_kernel_spmd(
        nc, in_maps, core_ids=list(range(N_CORES)), trace=_trace
    )

    out = np.zeros((B, T, D_MODEL), dtype=np.float32)
    for c in range(N_CORES):
        out[c // (N_CORES // B)] += res.results[c]["outp"]
    out += bo[None, None, :]
    if _trace:
        kernel.last_exec_time_ns = res.exec_time_ns
        kernel.last_res = res
    return out


# revision 12
# speedup vs baseline: 1.1440x; 1.1440x over previous
"""MultiHeadAttention (B=2, T=2048, D=512, H=8, causal) on 8 trn2 NeuronCores.

Sharding: batch*heads across cores. Core c handles batch c//4 and heads
{2*(c%4), 2*(c%4)+1}. Each core projects Q/K/V for its two heads (weight
slices replicated), runs softmax attention with scores materialized
transposed ([keys, queries] so the softmax reduction lands on the PSUM/matmul
path instead of cross-partition ops), applies its slice of the output
projection, and writes a [T, D] partial. Host sums the 8 partials (+ output
bias) into the full [B, T, D] result.

Numerics: all large matmuls run in float32r (TF32-like, ~1.2e-4 relative),
exact-fp32 PE transposes for V, softmax without max-subtraction (scores are
O(1) by construction: Q/K projections of unit-variance data through
U(-1/sqrt(D)) weights; exp stays far from fp32 range).
"""

import numpy as np

import concourse.bass as bass
import concourse.mybir as mybir
import concourse.tile as tile
from concourse.bass_utils import run_bass_kernel_spmd

D_MODEL = 512
N_HEADS = 8
D_K = 64
B = 2
T = 2048
N_CORES = 8
P = 128
QC = 4           # query chunks of 512
QW = T // QC     # 512 queries per chunk
KT = T // P      # 16 key tiles of 128
F32 = mybir.dt.float32
F32R = mybir.dt.float32r
BF16 = mybir.dt.bfloat16

# dtype configuration (module-level; set before first kernel() call)
ATTN_DT = "f32r"   # "f32r" | "bf16"  -- scores/AV operand dtype
IN_DT = "f32"      # "f32" | "bf16"   -- host->device x + projection dtype

_BUILD_CACHE = {}


def _hoist_excess_waits(nc, max_waits=1):
    """walrus codegen supports at most one sync-wait slot per hardware
    instruction, but Tile's sem-assignment can attach several (e.g. inputs
    arriving via two HW-DGE queues). Move the excess onto same-engine no-ops
    placed just before the instruction."""
    n_fixed = 0
    for fn in nc.m.functions:
        for bb in fn.blocks:
            insts = bb.instructions
            new_list = []
            for ins in insts:
                si = ins.sync_info
                ow = list(si.on_wait or []) if si else []
                if len(ow) > max_waits and ins.is_executable():
                    for j, w in enumerate(ow[max_waits:]):
                        nop = mybir.InstNoOp(
                            name=f"waitnop{j}_{ins.name}", ins=[], outs=[]
                        )
                        nop.engine = ins.engine
                        nop.sync_info = mybir.SyncInfo(on_wait=[w], on_update=[])
                        new_list.append(nop)
                    si.on_wait = ow[:max_waits]
                    ins.sync_info = si
                    n_fixed += 1
                new_list.append(ins)
            insts[:] = new_list
    return n_fixed


def _build_nc(plan, attn_dt, in_dt):
    """plan: 'causal' or 'full' -> one SPMD program for all 8 cores."""
    nc = bass.Bass(target_bir_lowering=False)
    ADT = BF16 if attn_dt == "bf16" else F32R
    XDT = BF16 if in_dt == "bf16" else F32
    XSB = BF16 if in_dt == "bf16" else F32R

    qT = nc.dram_tensor("qT", [P, 4, T], XDT, kind="ExternalInput")
    kT = nc.dram_tensor("kT", [P, 4, T], XDT, kind="ExternalInput")
    vT = nc.dram_tensor("vT", [P, 4, T], XDT, kind="ExternalInput")
    # wqkv: [128, 4(kc), 3(q/k/v), 128] interleaved on host
    wqkv = nc.dram_tensor("wqkv", [P, 4, 3, P], XDT, kind="ExternalInput")
    # misc: [128, 3+64+64] = biases (q,k,v) | idstack | ones
    misc = nc.dram_tensor("misc", [P, 3 + 2 * D_K], F32, kind="ExternalInput")
    # wo2: [128, 512] -- both heads' wo columns stacked on partitions
    wo2 = nc.dram_tensor("wo2", [P, D_MODEL], F32, kind="ExternalInput")
    outp = nc.dram_tensor("outp", [T, D_MODEL], F32, kind="ExternalOutput")

    def kts_of(qc):
        return list(range(KT if plan == "full" else 4 * (qc + 1)))

    def is_partial(qc, kt):
        return plan == "causal" and 4 * qc <= kt <= 4 * qc + 3

    with tile.TileContext(nc) as tc:
        with (
            tc.tile_pool(name="consts", bufs=1) as consts,
            tc.tile_pool(name="xin", bufs=1) as xin,
            tc.tile_pool(name="projT", bufs=1) as projT,
            tc.tile_pool(name="epool", bufs=6) as epool,
            tc.tile_pool(name="rpool", bufs=2) as rpool,
            tc.tile_pool(name="opool", bufs=3) as opool,
            tc.tile_pool(name="mm_ps", bufs=2, space="PSUM") as mm_ps,
            tc.tile_pool(name="s2_ps", bufs=2, space="PSUM") as s2_ps,
            tc.tile_pool(name="ctx_ps", bufs=2, space="PSUM") as ctx_ps_pool,
        ):
            # ---- constants (3 consolidated DMAs) ----
            wqkv_sb = consts.tile([P, 4, 3, P], XSB, tag="wqkv")
            nc.scalar.dma_start(wqkv_sb, wqkv.ap().bitcast(XSB))
            wq_sb = wqkv_sb[:, :, 0, :]
            wk_sb = wqkv_sb[:, :, 1, :]
            wv_sb = wqkv_sb[:, :, 2, :]
            misc_sb = consts.tile([P, 3 + 2 * D_K], F32, tag="misc")
            nc.sync.dma_start(misc_sb, misc[:, :])
            bq_sb = misc_sb[:, 0:1]
            bk_sb = misc_sb[:, 1:2]
            bv_sb = misc_sb[:, 2:3]
            ident = misc_sb[:, 3:3 + D_K]
            ones_r = consts.tile([P, D_K], F32R, tag="ones_r")
            nc.scalar.dma_start(ones_r, misc[:, 3 + D_K:3 + 2 * D_K].bitcast(F32R))
            wo_all = consts.tile([P, D_MODEL], F32R, tag="wo")
            nc.scalar.dma_start(wo_all, wo2.ap().bitcast(F32R))

            # ---- projections ----
            qhT = projT.tile([P, T], ADT, tag="qhT")   # [2*dk, T]
            khT = projT.tile([P, T], ADT, tag="khT")
            vhT = projT.tile([P, T], F32, tag="vhT")    # fp32: transposed exactly

            vaug = projT.tile([P, KT, 2, D_K + 1], ADT, tag="vaug")

            for h in range(2):
                nc.vector.tensor_copy(
                    vaug[:, :, h, D_K:D_K + 1],
                    ones_r[:, 0:1, None].to_broadcast((P, KT, 1)),
                )

            x_tiles = {}

            def load_x(xT_dram, xname):
                x_sb = xin.tile([P, 4, T], XSB, tag=f"x_{xname}", name=f"x_{xname}")
                for kc in range(4):
                    eng = nc.sync if kc % 2 == 0 else nc.scalar
                    eng.dma_start(
                        x_sb[:, kc, :], xT_dram.ap().bitcast(XSB)[:, kc, :]
                    )
                x_tiles[xname] = x_sb

            def project_chunk(xname, w_sb, b_sb, dst, qc):
                x_sb = x_tiles[xname]
                ps = mm_ps.tile([P, QW], F32, tag="mm", name=f"ps_{xname}_{qc}")
                for kc in range(4):
                    nc.tensor.matmul(
                        ps, w_sb[:, kc, :], x_sb[:, kc, qc * QW:(qc + 1) * QW],
                        start=(kc == 0), stop=(kc == 3),
                    )
                nc.vector.tensor_scalar(
                    out=dst[:, qc * QW:(qc + 1) * QW],
                    in0=ps, scalar1=b_sb, scalar2=None,
                    op0=mybir.AluOpType.add,
                )

            def vaug_chunk(qc):
                # transpose v-chunk qc's 4 key tiles into vaug (exact fp32)
                for kt in range(4 * qc, 4 * qc + 4):
                    for h in range(2):
                        tr = mm_ps.tile([P, D_K], F32, tag="mm", name=f"tr_{kt}_{h}")
                        nc.tensor.transpose(
                            tr,
                            vhT[h * D_K:(h + 1) * D_K, kt * P:(kt + 1) * P],
                            ident[h * D_K:(h + 1) * D_K, :],
                        )
                        nc.vector.tensor_copy(vaug[:, kt, h, 0:D_K], tr)

            # load whole tensors (128 x 8KB descriptors per DMA), then project
            load_x(kT, "k")
            load_x(qT, "q")
            load_x(vT, "v")
            for qc in range(QC):
                project_chunk("k", wk_sb, bk_sb, khT, qc)
            project_chunk("q", wq_sb, bq_sb, qhT, 3)
            for qc in range(QC):
                project_chunk("v", wv_sb, bv_sb, vhT, qc)
                vaug_chunk(qc)
            for qc in (2, 1, 0):
                project_chunk("q", wq_sb, bq_sb, qhT, qc)

            # ---- attention + output projection ----
            ctxT = projT.tile([P, T], F32R, tag="ctxT")

            for qc in (3, 2, 1, 0):
                kts = kts_of(qc)
                ctx_ps = [
                    ctx_ps_pool.tile([D_K + 1, QW], F32, tag="ctx", name=f"ctx_{qc}_{h}")
                    for h in range(2)
                ]
                def q_lo(kt):
                    # first valid query column within this chunk (diag trimming)
                    if plan != "causal":
                        return 0
                    return max(0, kt * P - qc * QW)

                for kp in range(0, len(kts), 2):
                    pair = kts[kp:kp + 2]
                    for h in range(2):
                        hs = slice(h * D_K, (h + 1) * D_K)
                        s_ps = s2_ps.tile([P, 2, QW], F32, tag="s2")
                        for j, kt in enumerate(pair):
                            lo = q_lo(kt)
                            nc.tensor.matmul(
                                s_ps[:, j, lo:],
                                khT[hs, kt * P:(kt + 1) * P],
                                qhT[hs, qc * QW + lo:(qc + 1) * QW],
                                start=True, stop=True,
                            )
                        e_sb = epool.tile([P, 2, QW], ADT, tag="e")
                        lo0 = q_lo(pair[0])
                        if len(pair) == 2 and q_lo(pair[1]) == lo0:
                            nc.scalar.activation(
                                e_sb[:, :, lo0:], s_ps[:, :, lo0:],
                                mybir.ActivationFunctionType.Exp,
                            )
                        else:
                            for j, kt in enumerate(pair):
                                lo = q_lo(kt)
                                nc.scalar.activation(
                                    e_sb[:, j, lo:], s_ps[:, j, lo:],
                                    mybir.ActivationFunctionType.Exp,
                                )
                        for j, kt in enumerate(pair):
                            lo = q_lo(kt)
                            if is_partial(qc, kt):
                                nc.gpsimd.affine_select(
                                    out=e_sb[:, j, lo:], in_=e_sb[:, j, lo:],
                                    compare_op=mybir.AluOpType.is_ge,
                                    fill=0.0,
                                    base=qc * QW + lo - kt * P,
                                    pattern=[[1, QW - lo]],
                                    channel_multiplier=-1,
                                )
                            nc.tensor.matmul(
                                ctx_ps[h][:, lo:],
                                vaug[:, kt, h, :],
                                e_sb[:, j, lo:],
                                start=(kt == kts[0]), stop=(kt == kts[-1]),
                            )
                for h in range(2):
                    recip = rpool.tile([D_K + 1, QW], F32R, tag="recip")
                    lnd = rpool.tile([D_K + 1, QW], F32, tag="lnd")
                    nc.scalar.activation(
                        lnd[D_K:D_K + 1, :], ctx_ps[h][D_K:D_K + 1, :],
                        mybir.ActivationFunctionType.Ln,
                    )
                    nc.scalar.activation(
                        recip[D_K:D_K + 1, :], lnd[D_K:D_K + 1, :],
                        mybir.ActivationFunctionType.Exp, scale=-1.0,
                    )
                    bc_ps = mm_ps.tile([D_K, QW], F32, tag="mm")
                    nc.tensor.matmul(
                        bc_ps,
                        ones_r[D_K:D_K + 1, :],
                        recip[D_K:D_K + 1, :],
                        start=True, stop=True,
                    )
                    bcast = rpool.tile([D_K, QW], F32R, tag="bcast")
                    nc.vector.tensor_copy(bcast, bc_ps)
                    nc.vector.tensor_tensor(
                        ctxT[h * D_K:(h + 1) * D_K, qc * QW:(qc + 1) * QW],
                        ctx_ps[h][0:D_K, :],
                        bcast,
                        mybir.AluOpType.mult,
                    )

                # O-projection for this query chunk (4 token tiles of 128)
                for qt in range(qc * 4, (qc + 1) * 4):
                    o_ps = mm_ps.tile([P, D_MODEL], F32, tag="mm")
                    nc.tensor.matmul(
                        o_ps, ctxT[:, qt * P:(qt + 1) * P], wo_all,
                        start=True, stop=True,
                    )
                    o_sb = opool.tile([P, D_MODEL], F32, tag="o")
                    nc.vector.tensor_copy(o_sb, o_ps)
                    nc.sync.dma_start(outp[qt * P:(qt + 1) * P, :], o_sb)

    _hoist_excess_waits(nc)
    return nc


def get_nc(plan):
    key = (plan, ATTN_DT, IN_DT)
    if key not in _BUILD_CACHE:
        _BUILD_CACHE[key] = _build_nc(plan, ATTN_DT, IN_DT)
    return _BUILD_CACHE[key]


def make_in_maps(q, k, v, wq, bq, wk, bk, wv, bv, wo):
    scale = 1.0 / np.sqrt(D_K)
    if IN_DT == "bf16":
        import ml_dtypes
        xdt = ml_dtypes.bfloat16
    else:
        xdt = np.float32
    idstack = np.concatenate([np.eye(D_K, dtype=np.float32)] * 2, axis=0)
    ones_in = np.ones((P, D_K), dtype=np.float32)
    def interleave(x):
        # [T, D] -> x.T grouped as [128, 4, T]: row kc*128+p of x.T at [p, kc]
        return np.ascontiguousarray(
            x.T.reshape(4, P, T).transpose(1, 0, 2)
        ).astype(xdt)

    def w_interleave(w):
        # [128 out, 512 in] -> lhsT chunks [128 p, 4 kc, 128 out]
        return w.T.reshape(4, P, P).transpose(1, 0, 2)

    in_maps = []
    for c in range(N_CORES):
        b = c // (N_CORES // B)
        h0 = 2 * (c % (N_CORES // B))
        ds = slice(h0 * D_K, (h0 + 2) * D_K)
        wqkv_arr = np.ascontiguousarray(np.stack([
            w_interleave(wq[ds] * scale),
            w_interleave(wk[ds]),
            w_interleave(wv[ds]),
        ], axis=2)).astype(xdt)
        misc_arr = np.ascontiguousarray(np.concatenate([
            (bq[ds] * scale).reshape(P, 1),
            bk[ds].reshape(P, 1),
            bv[ds].reshape(P, 1),
            idstack,
            ones_in,
        ], axis=1)).astype(np.float32)
        in_maps.append({
            "qT": interleave(q[b]),
            "kT": interleave(k[b]),
            "vT": interleave(v[b]),
            "wqkv": wqkv_arr,
            "misc": misc_arr,
            "wo2": np.ascontiguousarray(wo[:, ds].T),
        })
    return in_maps


def classify_mask(mask):
    m = np.asarray(mask)
    if m.all():
        return "full"
    tril = np.tril(np.ones((T, T), dtype=bool))
    if all(np.array_equal(m[b, 0], tril) for b in range(m.shape[0])):
        return "causal"
    return "general"


def _numpy_reference(q, k, v, mask, wq, bq, wk, bk, wv, bv, wo, bo):
    """Fallback for mask patterns the device program doesn't cover."""
    qh = (q @ wq.T + bq).reshape(B, T, N_HEADS, D_K).transpose(0, 2, 1, 3)
    kh = (k @ wk.T + bk).reshape(B, T, N_HEADS, D_K).transpose(0, 2, 1, 3)
    vh = (v @ wv.T + bv).reshape(B, T, N_HEADS, D_K).transpose(0, 2, 1, 3)
    s = np.einsum("bhqd,bhkd->bhqk", qh, kh) / np.sqrt(D_K).astype(np.float32)
    s = np.where(mask, s, -np.inf)
    all_masked = ~mask.any(axis=-1, keepdims=True)
    s = np.where(all_masked, 0.0, s)
    s = s - s.max(axis=-1, keepdims=True)
    e = np.exp(s)
    p = e / e.sum(axis=-1, keepdims=True)
    ctx = np.einsum("bhqk,bhkd->bhqd", p, vh)
    ctx = ctx.transpose(0, 2, 1, 3).reshape(B, T, D_MODEL)
    return (ctx @ wo.T + bo).astype(np.float32)


def kernel(q, k, v, mask, wq, bq, wk, bk, wv, bv, wo, bo, _trace=False):
    q, k, v = (np.asarray(x, dtype=np.float32) for x in (q, k, v))
    mask = np.asarray(mask, dtype=bool)
    wq, bq, wk, bk, wv, bv, wo, bo = (
        np.asarray(x, dtype=np.float32) for x in (wq, bq, wk, bk, wv, bv, wo, bo)
    )

    plan = classify_mask(mask)
    if plan == "general":
        return _numpy_reference(q, k, v, mask, wq, bq, wk, bk, wv, bv, wo, bo)

    nc = get_nc(plan)
    in_maps = make_in_maps(q, k, v, wq, bq, wk, bk, wv, bv, wo)
    res = run_bass_kernel_spmd(
        nc, in_maps, core_ids=list(range(N_CORES)), trace=_trace
    )

    out = np.zeros((B, T, D_MODEL), dtype=np.float32)
    for c in range(N_CORES):
        out[c // (N_CORES // B)] += res.results[c]["outp"]
    out += bo[None, None, :]
    if _trace:
        kernel.last_exec_time_ns = res.exec_time_ns
        kernel.last_res = res
    return out
